# revision 30
# baseline (speedup 1.0000x reference)
"""RBF (Gaussian) kernel Gram matrix on 8 Trainium2 NeuronCores.

out[i, j] = exp(-gamma * ||x_i - y_j||^2),  x, y: [8192, 256] fp32.

Strategy (2x4 block sharding, all engines near their roofline):
  - Cores arranged 2 (x-row groups) x 4 (y-col groups): each core owns a
    [4096, 2048] block of the Gram matrix. This minimizes per-core input
    DMA (xt 2MB + yt 1MB + y2b 0.5MB) vs the 1D row shard (4.5MB).
  - Per 128-row tile: PSUM accumulates 2 k=128 fp16 matmuls (x.T
    stationary, y.T streaming) -> pure GEMM; no k=2 fold-in matmuls
    (those stream full N cycles each and wasted 33% of PE time).
  - ACT: et_bf16 = exp(2*gamma*psum - gamma*||x_i||^2) straight out of
    PSUM (bias is the free per-partition affine). bf16 out: an fp16 ACT
    destination measures ~20% slower (2360ns vs 1967ns per [128,2048]).
    ACT is the bottleneck engine: it is the only unit that can drain
    PSUM and evaluate exp, at 1 elem/lane/cycle @ 1.2GHz = ~63us/core.
  - DVE: multiply by exp(-gamma*||y_j||^2) (host-precomputed bf16 row,
    broadcast to 128 partitions host-side) in 2x mode -> bf16 out tile.
  - bf16 output halves the dominant HBM write stream (16.8MB/core);
    host widens to fp32. Engine budget per core: ACT ~63us (bottleneck),
    PE ~57us, DMA ~57us, DVE ~39us. Measured ~87us vs 122us baseline.
"""

import numpy as np

GAMMA = 0.005
FULL_N = 8192
D = 256
N_CORES = 8
RX = 2  # x-row groups
RY = 4  # y-col groups
M_SHARD = FULL_N // RX  # 4096 rows of x per core
N_COLS = FULL_N // RY  # 2048 cols of y per core
P = 128
M_TILES = M_SHARD // P  # 32
BANK = 512  # fp32 columns per PSUM bank (one matmul's max free dim)

_cache = {}


def _split_sync_waits(nc, maxw=1):
    """walrus codegen rejects instructions carrying more than ~2 sync waits
    ("Too many sync wait commands"). Tile can attach many (e.g. the tail
    drain waits on every semaphore; a matmul can wait on several DMA lanes).
    Hoist the excess onto wait-only EventSemaphore instructions inserted
    just before the offender on the same engine (engines execute their
    instructions in block order, so all waits still precede the op)."""
    import concourse.mybir as mybir

    n_new = 0
    for fn in nc.m.functions:
        for bb in fn.blocks:
            insts = bb.instructions
            if not any(
                i.sync_info is not None and len(i.sync_info.on_wait) > maxw
                for i in insts
            ):
                continue
            new = []
            for inst in insts:
                si = inst.sync_info
                if si is not None and len(si.on_wait) > maxw:
                    waits = list(si.on_wait)
                    for i in range(0, len(waits) - maxw, maxw):
                        ev = mybir.InstEventSemaphore(
                            name=f"wsplit_{n_new}", ins=[], outs=[]
                        )
                        n_new += 1
                        ev.engine = inst.engine
                        ev.sync_info = mybir.SyncInfo(
                            on_wait=waits[i : i + maxw], on_update=[]
                        )
                        new.append(ev)
                    si.on_wait = waits[len(waits) - maxw :]
                new.append(inst)
            bb.instructions = new


def _build():
    import concourse.bass as bass
    import concourse.mybir as mybir
    import concourse.tile as tile

    f32 = mybir.dt.float32
    f16 = mybir.dt.float16
    bf16 = mybir.dt.bfloat16
    nc = bass.Bass("TRN2", target_bir_lowering=False, debug=False)
    xt = nc.dram_tensor("xt", [D, M_SHARD], f16, kind="ExternalInput").ap()
    yt = nc.dram_tensor("yt", [D, N_COLS], f16, kind="ExternalInput").ap()
    x2 = nc.dram_tensor("x2", [P, M_TILES], f32, kind="ExternalInput").ap()
    y2b = nc.dram_tensor("y2b", [P, N_COLS], bf16, kind="ExternalInput").ap()
    out = nc.dram_tensor("out", [M_SHARD, N_COLS], bf16, kind="ExternalOutput").ap()

    with tile.TileContext(nc) as tc:
        with (
            tc.tile_pool(name="const", bufs=1) as cpool,
            tc.tile_pool(name="ep", bufs=3) as epool,
            tc.tile_pool(name="outp", bufs=4) as opool,
            tc.tile_pool(name="psum", bufs=2, space="PSUM") as ppool,
        ):
            # Warm the ACT exp table at t=0: walrus puts the ~2.7us
            # PSEUDO_LOAD_ACT_FUNC_SET before the first ACTIVATE in the
            # Scalar stream, so an early dep-free activation overlaps the
            # table DMA with the input loads.
            # bias reuses the memset zero tile: a float bias would create a
            # const AP whose ~1us TENSOR_LOAD sits in the Sync preamble
            # ahead of the first input DMA.
            warm_in = cpool.tile([P, 1], f32, tag="warm_in")
            nc.vector.memset(warm_in, 0.0)
            warm_out = cpool.tile([P, 1], f32, tag="warm_out")
            nc.scalar.activation(
                warm_out, warm_in, mybir.ActivationFunctionType.Exp,
                bias=warm_in[:, 0:1],
            )

            # HWDGE DMAs drain FIFO per ring, so the Sync ring carries the
            # critical y tiles FIRST (nothing competes with them), then the
            # lower-priority bulk, then the output stream. The parallel
            # Scalar ring carries the tiny first-tile loads.
            xt0 = cpool.tile([P, M_SHARD], f16, tag="xt0")
            xt1 = cpool.tile([P, M_SHARD], f16, tag="xt1")
            nc.scalar.dma_start(out=xt0[:, 0:P], in_=xt[0:P, 0:P])
            nc.scalar.dma_start(out=xt1[:, 0:P], in_=xt[P : 2 * P, 0:P])
            x2sb = cpool.tile([P, M_TILES], f32, tag="x2")
            nc.scalar.dma_start(out=x2sb, in_=x2)
            # Whole-tile y loads: fewer, larger critical DMAs win — each
            # extra DMA on the critical path costs its own ~2us completion
            # receipt (measured: splitting yt into halves or interleaving
            # chunks delayed the first full-rate ACT by 2-6us).
            HALF = N_COLS // 2
            yt0 = cpool.tile([P, N_COLS], f16, tag="yt0")
            yt1 = cpool.tile([P, N_COLS], f16, tag="yt1")
            nc.sync.dma_start(out=yt0, in_=yt[0:P, :])
            nc.sync.dma_start(out=yt1, in_=yt[P : 2 * P, :])
            y2bsb = cpool.tile([P, N_COLS], bf16, tag="y2b")
            nc.sync.dma_start(out=y2bsb, in_=y2b)
            # Rest of x in per-few-tile chunks so tile t never waits on a
            # monolithic 1MB load; FIFO keeps these behind yt/y2b.
            for lo, hi in ((P, 512), (512, 1024), (1024, 2048), (2048, M_SHARD)):
                nc.sync.dma_start(out=xt0[:, lo:hi], in_=xt[0:P, lo:hi])
                nc.sync.dma_start(out=xt1[:, lo:hi], in_=xt[P : 2 * P, lo:hi])

            # PE HAM warmup: >=3.4us of sustained dummy matmuls on a memset
            # tile while the inputs stream in, so the 1.2->2.4GHz clock
            # un-throttle happens before the first real tile instead of
            # during it (and ends right as the y tiles land).
            warm_w = cpool.tile([P, 256], f16, tag="warm_w")
            nc.vector.memset(warm_w, 0.0)
            ps_warm = ppool.tile([P, N_COLS], f32, tag="ps")
            for _ in range(24):
                nc.tensor.matmul(
                    ps_warm[:, 0:256], warm_w[:, 0:P], warm_w,
                    start=True, stop=True,
                )

            for t in range(M_TILES):
                msl = slice(t * P, (t + 1) * P)
                ps = ppool.tile([P, N_COLS], f32, tag="ps")
                for d, (lhs_full, ytd) in enumerate(((xt0, yt0), (xt1, yt1))):
                    lhs = lhs_full[:, msl]
                    for b in range(N_COLS // BANK):
                        bsl = slice(b * BANK, (b + 1) * BANK)
                        nc.tensor.matmul(
                            ps[:, bsl], lhs, ytd[:, bsl],
                            start=(d == 0), stop=(d == 1),
                        )
                et = epool.tile([P, N_COLS], bf16, tag="et")
                # exp(2g*x.y - g*||x||^2), with the -g*||x||^2 bias
                # applied by the ACT affine for free.
                nc.scalar.activation(
                    et, ps, mybir.ActivationFunctionType.Exp,
                    bias=x2sb[:, t : t + 1], scale=2.0 * GAMMA,
                )
                if t < M_TILES - 1:
                    ot = opool.tile([P, N_COLS], bf16, tag="ot")
                    nc.vector.tensor_mul(ot, et, y2bsb)
                    nc.sync.dma_start(out=out[msl, :], in_=ot)
                else:
                    # Last tile: halve DVE/DMA so the final multiply and
                    # output DMA overlap instead of serializing.
                    for h in range(2):
                        hs = slice(h * HALF, (h + 1) * HALF)
                        ot = opool.tile([P, N_COLS], bf16, tag="ot")
                        nc.vector.tensor_mul(ot[:, hs], et[:, hs], y2bsb[:, hs])
                        nc.sync.dma_start(out=out[msl, hs], in_=ot[:, hs])

    _split_sync_waits(nc)
    return nc


def kernel(x: np.ndarray, y: np.ndarray) -> np.ndarray:
    from concourse import bass_utils

    x = np.asarray(x, dtype=np.float32)
    y = np.asarray(y, dtype=np.float32)

    if "nc" not in _cache:
        _cache["nc"] = _build()
    nc = _cache["nc"]

    import ml_dtypes

    xt_full = x.T.astype(np.float16)  # [256, 8192]
    yt_full = np.ascontiguousarray(y.T.astype(np.float16))  # [256, 8192]
    x2 = np.sum(x.astype(np.float64) * x.astype(np.float64), axis=1)  # [8192]
    y2 = np.sum(y.astype(np.float64) * y.astype(np.float64), axis=1)  # [8192]
    y2e = np.exp(-GAMMA * y2).astype(ml_dtypes.bfloat16)  # [8192]

    in_maps = []
    for c in range(N_CORES):
        cx, cy = c // RY, c % RY
        rows = slice(cx * M_SHARD, (cx + 1) * M_SHARD)
        cols = slice(cy * N_COLS, (cy + 1) * N_COLS)
        x2c = (-GAMMA * x2[rows]).astype(np.float32)
        in_maps.append(
            {
                "xt": np.ascontiguousarray(xt_full[:, rows]),
                "yt": np.ascontiguousarray(yt_full[:, cols]),
                "x2": np.ascontiguousarray(x2c.reshape(M_TILES, P).T),
                "y2b": np.ascontiguousarray(
                    np.broadcast_to(y2e[cols], (P, N_COLS))
                ),
            }
        )

    res = bass_utils.run_bass_kernel_spmd(
        nc, in_maps, core_ids=list(range(N_CORES))
    )
    _cache["last_result"] = res
    blocks = [
        [
            res.results[cx * RY + cy]["out"].astype(np.float32)
            for cy in range(RY)
        ]
        for cx in range(RX)
    ]
    return np.block(blocks)


# revision 34
# speedup vs baseline: 1.0160x; 1.0160x over previous
"""RBF (Gaussian) kernel Gram matrix on 8 Trainium2 NeuronCores.

out[i, j] = exp(-gamma * ||x_i - y_j||^2),  x, y: [8192, 256] fp32.

Strategy (2x4 block sharding, all engines near their roofline):
  - Cores arranged 2 (x-row groups) x 4 (y-col groups): each core owns a
    [4096, 2048] block of the Gram matrix. This minimizes per-core input
    DMA (xt 2MB + yt 1MB + y2b 0.5MB) vs the 1D row shard (4.5MB).
  - Per 128-row tile: PSUM accumulates 2 k=128 fp16 matmuls (x.T
    stationary, y.T streaming) -> pure GEMM; no k=2 fold-in matmuls
    (those stream full N cycles each and wasted 33% of PE time).
  - ACT: et_bf16 = exp(2*gamma*psum - gamma*||x_i||^2) straight out of
    PSUM (bias is the free per-partition affine). bf16 out: an fp16 ACT
    destination measures ~20% slower (2360ns vs 1967ns per [128,2048]).
    ACT is the bottleneck engine: it is the only unit that can drain
    PSUM and evaluate exp, at 1 elem/lane/cycle @ 1.2GHz = ~63us/core.
  - DVE: multiply by exp(-gamma*||y_j||^2) (host-precomputed bf16 row,
    broadcast to 128 partitions host-side) in 2x mode -> bf16 out tile.
  - bf16 output halves the dominant HBM write stream (16.8MB/core);
    host widens to fp32. Engine budget per core: ACT ~63us (bottleneck),
    PE ~57us, DMA ~57us, DVE ~39us. Measured ~87us vs 122us baseline.
"""

import numpy as np

GAMMA = 0.005
FULL_N = 8192
D = 256
N_CORES = 8
RX = 2  # x-row groups
RY = 4  # y-col groups
M_SHARD = FULL_N // RX  # 4096 rows of x per core
N_COLS = FULL_N // RY  # 2048 cols of y per core
P = 128
M_TILES = M_SHARD // P  # 32
BANK = 512  # fp32 columns per PSUM bank (one matmul's max free dim)

_cache = {}


def _split_sync_waits(nc, maxw=1):
    """walrus codegen rejects instructions carrying more than ~2 sync waits
    ("Too many sync wait commands"). Tile can attach many (e.g. the tail
    drain waits on every semaphore; a matmul can wait on several DMA lanes).
    Hoist the excess onto wait-only EventSemaphore instructions inserted
    just before the offender on the same engine (engines execute their
    instructions in block order, so all waits still precede the op)."""
    import concourse.mybir as mybir

    n_new = 0
    for fn in nc.m.functions:
        for bb in fn.blocks:
            insts = bb.instructions
            if not any(
                i.sync_info is not None and len(i.sync_info.on_wait) > maxw
                for i in insts
            ):
                continue
            new = []
            for inst in insts:
                si = inst.sync_info
                if si is not None and len(si.on_wait) > maxw:
                    waits = list(si.on_wait)
                    for i in range(0, len(waits) - maxw, maxw):
                        ev = mybir.InstEventSemaphore(
                            name=f"wsplit_{n_new}", ins=[], outs=[]
                        )
                        n_new += 1
                        ev.engine = inst.engine
                        ev.sync_info = mybir.SyncInfo(
                            on_wait=waits[i : i + maxw], on_update=[]
                        )
                        new.append(ev)
                    si.on_wait = waits[len(waits) - maxw :]
                new.append(inst)
            bb.instructions = new


def _build():
    import concourse.bass as bass
    import concourse.mybir as mybir
    import concourse.tile as tile

    f32 = mybir.dt.float32
    f16 = mybir.dt.float16
    bf16 = mybir.dt.bfloat16
    nc = bass.Bass("TRN2", target_bir_lowering=False, debug=False)
    xt = nc.dram_tensor("xt", [D, M_SHARD], f16, kind="ExternalInput").ap()
    # Both k=128 row-blocks of y.T, host-interleaved side by side so ONE
    # DMA (one issue, one completion receipt) covers the whole critical
    # y load: ytb[p, 0:N] = y.T[p, :], ytb[p, N:2N] = y.T[p+128, :].
    ytb = nc.dram_tensor("ytb", [P, 2 * N_COLS], f16, kind="ExternalInput").ap()
    x2 = nc.dram_tensor("x2", [P, M_TILES], f32, kind="ExternalInput").ap()
    y2b = nc.dram_tensor("y2b", [P, N_COLS], bf16, kind="ExternalInput").ap()
    out = nc.dram_tensor("out", [M_SHARD, N_COLS], bf16, kind="ExternalOutput").ap()

    with tile.TileContext(nc) as tc:
        with (
            tc.tile_pool(name="const", bufs=1) as cpool,
            tc.tile_pool(name="ep", bufs=3) as epool,
            tc.tile_pool(name="outp", bufs=4) as opool,
            tc.tile_pool(name="psum", bufs=2, space="PSUM") as ppool,
        ):
            # Warm the ACT exp table at t=0: walrus puts the ~2.7us
            # PSEUDO_LOAD_ACT_FUNC_SET before the first ACTIVATE in the
            # Scalar stream, so an early dep-free activation overlaps the
            # table DMA with the input loads.
            # bias reuses the memset zero tile: a float bias would create a
            # const AP whose ~1us TENSOR_LOAD sits in the Sync preamble
            # ahead of the first input DMA.
            warm_in = cpool.tile([P, 1], f32, tag="warm_in")
            nc.vector.memset(warm_in, 0.0)
            warm_out = cpool.tile([P, 1], f32, tag="warm_out")
            nc.scalar.activation(
                warm_out, warm_in, mybir.ActivationFunctionType.Exp,
                bias=warm_in[:, 0:1],
            )

            # HWDGE DMAs drain FIFO per ring, so the Sync ring carries the
            # critical y tiles FIRST (nothing competes with them), then the
            # lower-priority bulk, then the output stream. The parallel
            # Scalar ring carries the tiny first-tile loads.
            xt0 = cpool.tile([P, M_SHARD], f16, tag="xt0")
            xt1 = cpool.tile([P, M_SHARD], f16, tag="xt1")
            nc.scalar.dma_start(out=xt0[:, 0:P], in_=xt[0:P, 0:P])
            nc.scalar.dma_start(out=xt1[:, 0:P], in_=xt[P : 2 * P, 0:P])
            x2sb = cpool.tile([P, M_TILES], f32, tag="x2")
            nc.scalar.dma_start(out=x2sb, in_=x2)
            # Whole y load as ONE DMA: fewer, larger critical DMAs win —
            # each extra DMA on the critical path costs its own issue slot
            # and ~2us completion receipt (measured: splitting yt into
            # halves or slivers delayed the first full-rate ACT by 2-6us).
            HALF = N_COLS // 2
            ytsb = cpool.tile([P, 2 * N_COLS], f16, tag="ytb")
            nc.sync.dma_start(out=ytsb, in_=ytb)
            y2bsb = cpool.tile([P, N_COLS], bf16, tag="y2b")
            nc.sync.dma_start(out=y2bsb, in_=y2b)
            # Rest of x in per-few-tile chunks so tile t never waits on a
            # monolithic 1MB load; FIFO keeps these behind yt/y2b.
            for lo, hi in ((P, 512), (512, 1024), (1024, 2048), (2048, M_SHARD)):
                nc.sync.dma_start(out=xt0[:, lo:hi], in_=xt[0:P, lo:hi])
                nc.sync.dma_start(out=xt1[:, lo:hi], in_=xt[P : 2 * P, lo:hi])

            # PE HAM warmup: >=3.4us of sustained dummy matmuls on a memset
            # tile while the inputs stream in, so the 1.2->2.4GHz clock
            # un-throttle happens before the first real tile instead of
            # during it (and ends right as the y tiles land).
            warm_w = cpool.tile([P, 256], f16, tag="warm_w")
            nc.vector.memset(warm_w, 0.0)
            ps_warm = ppool.tile([P, N_COLS], f32, tag="ps")
            for _ in range(24):
                nc.tensor.matmul(
                    ps_warm[:, 0:256], warm_w[:, 0:P], warm_w,
                    start=True, stop=True,
                )

            for t in range(M_TILES):
                msl = slice(t * P, (t + 1) * P)
                ps = ppool.tile([P, N_COLS], f32, tag="ps")
                for d, lhs_full in enumerate((xt0, xt1)):
                    lhs = lhs_full[:, msl]
                    for b in range(N_COLS // BANK):
                        bsl = slice(b * BANK, (b + 1) * BANK)
                        ysl = slice(
                            d * N_COLS + b * BANK, d * N_COLS + (b + 1) * BANK
                        )
                        nc.tensor.matmul(
                            ps[:, bsl], lhs, ytsb[:, ysl],
                            start=(d == 0), stop=(d == 1),
                        )
                et = epool.tile([P, N_COLS], bf16, tag="et")
                # exp(2g*x.y - g*||x||^2), with the -g*||x||^2 bias
                # applied by the ACT affine for free.
                nc.scalar.activation(
                    et, ps, mybir.ActivationFunctionType.Exp,
                    bias=x2sb[:, t : t + 1], scale=2.0 * GAMMA,
                )
                if t < M_TILES - 1:
                    ot = opool.tile([P, N_COLS], bf16, tag="ot")
                    nc.vector.tensor_mul(ot, et, y2bsb)
                    nc.sync.dma_start(out=out[msl, :], in_=ot)
                else:
                    # Last tile: halve DVE/DMA so the final multiply and
                    # output DMA overlap instead of serializing.
                    for h in range(2):
                        hs = slice(h * HALF, (h + 1) * HALF)
                        ot = opool.tile([P, N_COLS], bf16, tag="ot")
                        nc.vector.tensor_mul(ot[:, hs], et[:, hs], y2bsb[:, hs])
                        nc.sync.dma_start(out=out[msl, hs], in_=ot[:, hs])

    _split_sync_waits(nc)
    return nc


def kernel(x: np.ndarray, y: np.ndarray) -> np.ndarray:
    from concourse import bass_utils

    x = np.asarray(x, dtype=np.float32)
    y = np.asarray(y, dtype=np.float32)

    if "nc" not in _cache:
        _cache["nc"] = _build()
    nc = _cache["nc"]

    import ml_dtypes

    xt_full = x.T.astype(np.float16)  # [256, 8192]
    yt_full = np.ascontiguousarray(y.T.astype(np.float16))  # [256, 8192]
    x2 = np.sum(x.astype(np.float64) * x.astype(np.float64), axis=1)  # [8192]
    y2 = np.sum(y.astype(np.float64) * y.astype(np.float64), axis=1)  # [8192]
    y2e = np.exp(-GAMMA * y2).astype(ml_dtypes.bfloat16)  # [8192]

    in_maps = []
    for c in range(N_CORES):
        cx, cy = c // RY, c % RY
        rows = slice(cx * M_SHARD, (cx + 1) * M_SHARD)
        cols = slice(cy * N_COLS, (cy + 1) * N_COLS)
        x2c = (-GAMMA * x2[rows]).astype(np.float32)
        yts = yt_full[:, cols]  # [256, N_COLS]
        in_maps.append(
            {
                "xt": np.ascontiguousarray(xt_full[:, rows]),
                "ytb": np.ascontiguousarray(
                    np.concatenate([yts[0:P, :], yts[P : 2 * P, :]], axis=1)
                ),
                "x2": np.ascontiguousarray(x2c.reshape(M_TILES, P).T),
                "y2b": np.ascontiguousarray(
                    np.broadcast_to(y2e[cols], (P, N_COLS))
                ),
            }
        )

    res = bass_utils.run_bass_kernel_spmd(
        nc, in_maps, core_ids=list(range(N_CORES))
    )
    _cache["last_result"] = res
    blocks = [
        [
            res.results[cx * RY + cy]["out"].astype(np.float32)
            for cy in range(RY)
        ]
        for cx in range(RX)
    ]
    return np.block(blocks)
